# revision 1
# baseline (speedup 1.0000x reference)
"""Llama GQA attention block, tensor-parallel over heads across 8 TRN2 NeuronCores.

Contract: kernel(**inputs) takes the FULL inputs of the reference
(x, freq_cos, freq_sin, w_q_w, w_q_b, w_kv_w, w_kv_b, proj_w, proj_b, start_pos)
and returns the FULL output (B, T, N_EMBD) float32.

Sharding: core c owns query heads 4c..4c+3 and KV head c, plus proj columns
c*512..(c+1)*512. Each core computes a partial projection output; the host sums
the 8 partials and adds proj_b.
"""

import math
import numpy as np
from contextlib import ExitStack

# Problem constants (hardcoded per the harness contract).
B = 2
T = 2048
E = 4096
D = 128          # head dim
NCORES = 8
HPC = 4          # query heads per core
BT = B * T       # 4096
SQ = 512         # token chunk (matmul moving dim)
ECH = E // 128   # 32 contraction chunks
CPB = T // SQ    # 4 tok chunks per batch
INV_SQRT_D = 1.0 / math.sqrt(D)
HB = 1024        # half-batch token span for the output projection stage


def _build_program():
    import concourse.bass as bass  # noqa: F401
    import concourse.mybir as mybir
    import concourse.tile as tile
    from concourse import bacc

    f32 = mybir.dt.float32
    f32r = mybir.dt.float32r
    AF = mybir.ActivationFunctionType

    nc = bacc.Bacc("TRN2", target_bir_lowering=False, debug=False)

    xT_d = nc.dram_tensor("xT", [E, BT], f32r, kind="ExternalInput")
    wq_d = nc.dram_tensor("wqT", [E, HPC * D], f32r, kind="ExternalInput")
    wkv_d = nc.dram_tensor("wkvT", [E, 2 * D], f32r, kind="ExternalInput")
    bias_d = nc.dram_tensor("biases", [6, 128], f32, kind="ExternalInput")
    cos_d = nc.dram_tensor("cosE", [128, T], f32, kind="ExternalInput")
    sin_d = nc.dram_tensor("sinS", [128, T], f32, kind="ExternalInput")
    mask_d = nc.dram_tensor("maskM", [128, 896], f32, kind="ExternalInput")
    pjt_d = nc.dram_tensor("projT", [HPC * D, E], f32r, kind="ExternalInput")
    idn_d = nc.dram_tensor("ident", [128, 128], f32, kind="ExternalInput")
    one_d = nc.dram_tensor("onescol", [128, 1], f32r, kind="ExternalInput")
    out_d = nc.dram_tensor("yp", [BT, E], f32, kind="ExternalOutput")

    with tile.TileContext(nc) as tc, ExitStack() as ctx:
        const = ctx.enter_context(tc.tile_pool(name="const", bufs=1))
        wpool = ctx.enter_context(tc.tile_pool(name="wpool", bufs=1))
        xpool = ctx.enter_context(tc.tile_pool(name="xpool", bufs=3))
        spool = ctx.enter_context(tc.tile_pool(name="spool", bufs=2))
        kvpool = ctx.enter_context(tc.tile_pool(name="kvpool", bufs=1))
        ypool = ctx.enter_context(tc.tile_pool(name="ypool", bufs=1))
        pjpool = ctx.enter_context(tc.tile_pool(name="pjpool", bufs=1))
        drpool = ctx.enter_context(tc.tile_pool(name="drpool", bufs=2, space="DRAM"))
        psum = ctx.enter_context(tc.tile_pool(name="ps", bufs=1, space="PSUM"))

        # ---- constants / weights resident in SBUF ----
        wq_sb = wpool.tile([128, ECH, HPC * D], f32r, tag="wq")
        for e in range(ECH):
            nc.sync.dma_start(wq_sb[:, e, :], wq_d[e * 128:(e + 1) * 128, :])
        wkv_sb = wpool.tile([128, ECH, 2 * D], f32r, tag="wkv")
        for e in range(ECH):
            nc.sync.dma_start(wkv_sb[:, e, :], wkv_d[e * 128:(e + 1) * 128, :])
        bias_sb = const.tile([128, 6], f32, tag="bias")
        nc.sync.dma_start(bias_sb[:], bias_d.rearrange("r p -> p r"))
        mask_sb = const.tile([128, 896], f32, tag="mask")
        nc.sync.dma_start(mask_sb[:], mask_d[:, :])
        idn_sb = const.tile([128, 128], f32, tag="idn")
        nc.sync.dma_start(idn_sb[:], idn_d[:, :])
        ones_sb = const.tile([128, 1], f32r, tag="ones")
        nc.sync.dma_start(ones_sb[:], one_d[:, :])

        ps_tag = [f"b{i}" for i in range(8)]
        tp_alt = 0      # alternator for transpose/score banks (b6/b7)
        yt_alt = 0      # alternator for YT banks (b4/b5)
        sm_alt = 0      # alternator for sums banks (b2/b3)
        po_alt = 0      # alternator for proj-out banks (b0/b1)

        for b in range(B):
            rotK = kvpool.tile([128, T], f32r, tag="rotK")
            vbuf = kvpool.tile([128, T], f32r, tag="vbuf")
            for hh in range(2):  # half-batches for the proj stage
                yts = [ypool.tile([128, HB], f32r, tag=f"yt{h}", name=f"yt{h}_{b}_{hh}")
                       for h in range(HPC)]
                for jj in range(2):  # tok chunks within half-batch
                    j = hh * 2 + jj          # chunk index within batch
                    gcol = b * T + j * SQ    # column into xT / row into out
                    tcol = j * SQ            # position within batch (rope/causal)

                    # ---- stage A: Q0..Q3, K, V projections ----
                    acc = [psum.tile([128, SQ], f32, tag=ps_tag[i],
                                     name=f"acc{i}_{b}_{j}") for i in range(6)]
                    for e in range(ECH):
                        xt = xpool.tile([128, SQ], f32r, tag="xt")
                        nc.sync.dma_start(xt[:], xT_d[e * 128:(e + 1) * 128,
                                                      gcol:gcol + SQ])
                        st, sp = (e == 0), (e == ECH - 1)
                        for h in range(HPC):
                            nc.tensor.matmul(
                                acc[h][:], wq_sb[:, e, h * D:(h + 1) * D],
                                xt[:], start=st, stop=sp)
                        nc.tensor.matmul(acc[4][:], wkv_sb[:, e, 0:D],
                                         xt[:], start=st, stop=sp)
                        nc.tensor.matmul(acc[5][:], wkv_sb[:, e, D:2 * D],
                                         xt[:], start=st, stop=sp)

                    # ---- rope tables for this chunk ----
                    cos_t = spool.tile([128, SQ], f32, tag="cos")
                    nc.sync.dma_start(cos_t[:], cos_d[:, tcol:tcol + SQ])
                    sin_t = spool.tile([128, SQ], f32, tag="sin")
                    nc.sync.dma_start(sin_t[:], sin_d[:, tcol:tcol + SQ])

                    def rope(ps, bias_col, out_ap):
                        raw = spool.tile([128, SQ], f32, tag="raw", bufs=3,
                                         name=f"raw_{b}_{j}_{bias_col}")
                        nc.scalar.activation(raw[:], ps[:], AF.Identity,
                                             bias=bias_sb[:, bias_col:bias_col + 1])
                        sw = spool.tile([128, SQ], f32, tag="sw", bufs=3,
                                        name=f"sw_{b}_{j}_{bias_col}")
                        raw3 = raw.rearrange("(a two) t -> a two t", two=2)
                        sw3 = sw.rearrange("(a two) t -> a two t", two=2)
                        nc.sync.dma_start(sw3[:, 1, :], raw3[:, 0, :])
                        nc.sync.dma_start(sw3[:, 0, :], raw3[:, 1, :])
                        nc.vector.tensor_mul(out_ap, raw[:], cos_t[:])
                        nc.vector.tensor_mul(sw[:], sw[:], sin_t[:])
                        nc.vector.tensor_add(out_ap, out_ap, sw[:])

                    qrots = []
                    for h in range(HPC):
                        qr_t = spool.tile([128, SQ], f32r, tag=f"qrot{h}",
                                          name=f"qrot{h}_{b}_{j}", bufs=1)
                        rope(acc[h], h, qr_t[:])
                        qrots.append(qr_t)
                    rope(acc[4], 4, rotK[:, tcol:tcol + SQ])

                    # ---- V: evict with bias then transpose to [tok, d] ----
                    vraw = spool.tile([128, SQ], f32, tag="raw", bufs=3,
                                      name=f"vraw_{b}_{j}")
                    nc.scalar.activation(vraw[:], acc[5][:], AF.Identity,
                                         bias=bias_sb[:, 5:6])
                    for t4 in range(4):
                        tp = psum.tile([128, 128], f32, tag=ps_tag[6 + tp_alt],
                                       name=f"tp_{b}_{j}_{t4}")
                        tp_alt ^= 1
                        nc.tensor.transpose(tp[:], vraw[:, t4 * 128:(t4 + 1) * 128],
                                            idn_sb[:])
                        nc.scalar.copy(vbuf[:, tcol + t4 * 128:tcol + (t4 + 1) * 128],
                                       tp[:])

                    # ---- attention for this 512-wide q tile ----
                    nkc = 4 * j + 4
                    for h in range(HPC):
                        yt_ps = psum.tile([128, SQ], f32, tag=ps_tag[4 + yt_alt],
                                          name=f"ytps_{b}_{j}_{h}")
                        yt_alt ^= 1
                        sm_ps = psum.tile([1, SQ], f32, tag=ps_tag[2 + sm_alt],
                                          name=f"smps_{b}_{j}_{h}")
                        sm_alt ^= 1
                        for kc in range(nkc):
                            s_ps = psum.tile([128, SQ], f32,
                                             tag=ps_tag[6 + tp_alt],
                                             name=f"sps_{b}_{j}_{h}_{kc}")
                            tp_alt ^= 1
                            nc.tensor.matmul(
                                s_ps[:], rotK[:, kc * 128:(kc + 1) * 128],
                                qrots[h][:], start=True, stop=True)
                            es = spool.tile([128, SQ], f32r, tag="es", bufs=4)
                            nc.scalar.activation(es[:], s_ps[:], AF.Exp,
                                                 scale=INV_SQRT_D)
                            if kc >= nkc - 4:
                                mj = kc - (nkc - 4)
                                off = (3 - mj) * 128
                                nc.vector.tensor_mul(es[:], es[:],
                                                     mask_sb[:, off:off + SQ])
                            st, sp = (kc == 0), (kc == nkc - 1)
                            nc.tensor.matmul(
                                yt_ps[:], vbuf[:, kc * 128:(kc + 1) * 128],
                                es[:], start=st, stop=sp)
                            nc.tensor.matmul(sm_ps[:], ones_sb[:], es[:],
                                             start=st, stop=sp)
                        rr = spool.tile([1, SQ], f32, tag="rr")
                        nc.vector.reciprocal(rr[:], sm_ps[:])
                        dr = drpool.tile([1, SQ], f32, tag="dr",
                                         name=f"dr_{b}_{j}_{h}")
                        nc.sync.dma_start(dr[:], rr[:])
                        rb = spool.tile([128, SQ], f32, tag="rb", bufs=3)
                        nc.sync.dma_start(rb[:], dr.to_broadcast((128, SQ)))
                        nc.vector.tensor_mul(yts[h][:, jj * SQ:(jj + 1) * SQ],
                                             yt_ps[:], rb[:])

                # ---- output projection for this half-batch (1024 toks) ----
                grow = b * T + hh * HB
                for oc in range(8):
                    pjs = []
                    for h in range(HPC):
                        pj = pjpool.tile([128, SQ], f32r, tag=f"pj{h}",
                                         name=f"pj{h}_{b}_{hh}_{oc}")
                        nc.sync.dma_start(pj[:], pjt_d[h * 128:(h + 1) * 128,
                                                       oc * SQ:(oc + 1) * SQ])
                        pjs.append(pj)
                    for ts8 in range(HB // 128):
                        po_ps = psum.tile([128, SQ], f32, tag=ps_tag[po_alt],
                                          name=f"pops_{b}_{hh}_{oc}_{ts8}")
                        po_alt ^= 1
                        for h in range(HPC):
                            nc.tensor.matmul(
                                po_ps[:], yts[h][:, ts8 * 128:(ts8 + 1) * 128],
                                pjs[h][:], start=(h == 0), stop=(h == HPC - 1))
                        po = spool.tile([128, SQ], f32, tag="po")
                        nc.scalar.copy(po[:], po_ps[:])
                        nc.sync.dma_start(
                            out_d[grow + ts8 * 128:grow + (ts8 + 1) * 128,
                                  oc * SQ:(oc + 1) * SQ], po[:])

    nc.compile()
    return nc


_PROG = None


def kernel(x, freq_cos, freq_sin, w_q_w, w_q_b, w_kv_w, w_kv_b, proj_w, proj_b,
           start_pos=0, **_unused):
    global _PROG
    from concourse.bass_utils import run_bass_kernel_spmd

    x = np.asarray(x, np.float32)
    freq_cos = np.asarray(freq_cos, np.float32)
    freq_sin = np.asarray(freq_sin, np.float32)
    w_q_w = np.asarray(w_q_w, np.float32)
    w_q_b = np.asarray(w_q_b, np.float32)
    w_kv_w = np.asarray(w_kv_w, np.float32)
    w_kv_b = np.asarray(w_kv_b, np.float32)
    proj_w = np.asarray(proj_w, np.float32)
    proj_b = np.asarray(proj_b, np.float32)

    xT = np.ascontiguousarray(x.reshape(BT, E).T)

    cosE = np.repeat(freq_cos.T, 2, axis=0).astype(np.float32)        # [128, T]
    sinE = np.repeat(freq_sin.T, 2, axis=0).astype(np.float32)
    sinS = sinE.copy()
    sinS[0::2, :] *= -1.0                                             # even rows -sin

    kp = np.arange(128)[:, None]
    cc = np.arange(896)[None, :]
    maskM = (cc >= kp + 384).astype(np.float32)

    ident = np.eye(128, dtype=np.float32)

    if _PROG is None:
        _PROG = _build_program()

    in_maps = []
    for c in range(NCORES):
        wq_c = np.ascontiguousarray(w_q_w[c * 512:(c + 1) * 512, :].T)   # [E, 512]
        kT = w_kv_w[c * D:(c + 1) * D, :].T                               # [E, 128]
        vT = w_kv_w[8 * D + c * D:8 * D + (c + 1) * D, :].T
        wkv_c = np.ascontiguousarray(np.concatenate([kT, vT], axis=1))    # [E, 256]
        biases = np.zeros((6, 128), np.float32)
        biases[0:4, :] = w_q_b[c * 512:(c + 1) * 512].reshape(4, 128)
        biases[4, :] = w_kv_b[c * D:(c + 1) * D]
        biases[5, :] = w_kv_b[8 * D + c * D:8 * D + (c + 1) * D]
        pjt_c = np.ascontiguousarray(proj_w[:, c * 512:(c + 1) * 512].T)  # [512, E]
        in_maps.append({
            "xT": xT, "wqT": wq_c, "wkvT": wkv_c, "biases": biases,
            "cosE": cosE, "sinS": sinS, "maskM": maskM, "projT": pjt_c,
            "ident": ident, "onescol": np.ones((128, 1), np.float32),
        })

    res = run_bass_kernel_spmd(_PROG, in_maps, core_ids=list(range(NCORES)))
    out = np.zeros((BT, E), np.float64)
    for c in range(NCORES):
        out += res.results[c]["yp"].astype(np.float64)
    out = out.astype(np.float32) + proj_b[None, :]
    return out.reshape(B, T, E)



# revision 2
# speedup vs baseline: 1.4925x; 1.4925x over previous
"""Llama GQA attention block, tensor-parallel over heads across 8 TRN2 NeuronCores.

Contract: kernel(**inputs) takes the FULL inputs of the reference
(x, freq_cos, freq_sin, w_q_w, w_q_b, w_kv_w, w_kv_b, proj_w, proj_b, start_pos)
and returns the FULL output (B, T, N_EMBD) float32.

Sharding: core c owns query heads 4c..4c+3 and KV head c, plus proj columns
c*512..(c+1)*512. Each core computes a partial projection output in bf16; the
host sums the 8 partials and adds proj_b.

v2 design notes (all-bf16 matmuls, fp32 PSUM accumulation):
- All weights (wq, wkv, proj) and rope tables resident in SBUF for the whole
  kernel; x streamed per 512-token chunk in two contiguous [128,16,512] halves.
- Stage A runs acc-major (K, V, Q0..Q3 each a dense 32-matmul accumulation)
  so the rope chains for early accumulators overlap the later matmul runs.
- RoPE pair-swap via DVE stream_shuffle (no DMA round trips).
- V transposed into [tok, d] blocks via DMA xbar transpose (off the PE).
- Causal diagonal 128-blocks are column-trimmed on score/PV/sum matmuls.
- Softmax denominator reciprocal broadcast via GpSimd partition_broadcast.
"""

import math
import numpy as np
from contextlib import ExitStack

# Problem constants (hardcoded per the harness contract).
B = 2
T = 2048
E = 4096
D = 128          # head dim
NCORES = 8
HPC = 4          # query heads per core
BT = B * T       # 4096
SQ = 512         # token chunk (matmul moving dim)
ECH = 32         # contraction chunks of 128 over E
CPB = T // SQ    # 4 tok chunks per batch
INV_SQRT_D = 1.0 / math.sqrt(D)
HB = 1024        # half-batch token span for the output projection stage

SWAP_MASK = [i ^ 1 for i in range(32)]  # pair swap within each 32-partition group


def _build_program():
    import concourse.bass as bass  # noqa: F401
    import concourse.mybir as mybir
    import concourse.tile as tile
    from concourse import bacc

    f32 = mybir.dt.float32
    bf16 = mybir.dt.bfloat16
    AF = mybir.ActivationFunctionType

    nc = bacc.Bacc("TRN2", target_bir_lowering=False, debug=False)

    xP_d = nc.dram_tensor("xP", [B * CPB * 2 * 128, 16 * SQ], bf16,
                          kind="ExternalInput")
    wq_d = nc.dram_tensor("wqP", [128, ECH * 512], bf16, kind="ExternalInput")
    wkv_d = nc.dram_tensor("wkvP", [128, ECH * 256], bf16, kind="ExternalInput")
    pjt_d = nc.dram_tensor("pjtP", [128, HPC * E], bf16, kind="ExternalInput")
    cos_d = nc.dram_tensor("cosE", [128, T], bf16, kind="ExternalInput")
    sin_d = nc.dram_tensor("sinS", [128, T], bf16, kind="ExternalInput")
    bias_d = nc.dram_tensor("biasP", [128, 6], f32, kind="ExternalInput")
    tri_d = nc.dram_tensor("triM", [128, 128], bf16, kind="ExternalInput")
    ones_d = nc.dram_tensor("onesC", [128, 1], bf16, kind="ExternalInput")
    out_d = nc.dram_tensor("yp", [BT, E], bf16, kind="ExternalOutput")

    with tile.TileContext(nc) as tc, ExitStack() as ctx:
        const = ctx.enter_context(tc.tile_pool(name="const", bufs=1))
        wpool = ctx.enter_context(tc.tile_pool(name="wpool", bufs=1))
        xpool = ctx.enter_context(tc.tile_pool(name="xpool", bufs=3))
        kvpool = ctx.enter_context(tc.tile_pool(name="kvpool", bufs=2))
        qpool = ctx.enter_context(tc.tile_pool(name="qpool", bufs=2))
        spool = ctx.enter_context(tc.tile_pool(name="spool", bufs=2))
        ypool = ctx.enter_context(tc.tile_pool(name="ypool", bufs=1))
        psum = ctx.enter_context(tc.tile_pool(name="ps", bufs=1, space="PSUM"))

        # ---- resident weights / constants ----
        wkv_sb = wpool.tile([128, ECH, 256], bf16, tag="wkv")
        nc.sync.dma_start(wkv_sb.rearrange("p c n -> p (c n)"), wkv_d[:, :])
        bias_sb = const.tile([128, 6], f32, tag="bias")
        nc.sync.dma_start(bias_sb[:], bias_d[:, :])
        wq_sb = wpool.tile([128, ECH, 512], bf16, tag="wq")
        nc.sync.dma_start(wq_sb.rearrange("p c n -> p (c n)"), wq_d[:, :])
        cos_sb = const.tile([128, T], bf16, tag="cos")
        nc.sync.dma_start(cos_sb[:], cos_d[:, :])
        sin_sb = const.tile([128, T], bf16, tag="sin")
        nc.sync.dma_start(sin_sb[:], sin_d[:, :])
        tri_sb = const.tile([128, 128], bf16, tag="tri")
        nc.sync.dma_start(tri_sb[:], tri_d[:, :])
        ones_sb = const.tile([128, 1], bf16, tag="ones")
        nc.sync.dma_start(ones_sb[:], ones_d[:, :])
        pjt_sb = wpool.tile([128, HPC, E], bf16, tag="pjt")
        nc.sync.dma_start(pjt_sb.rearrange("p h n -> p (h n)"), pjt_d[:, :])

        # Pre-warm the exp activation table while initial DMAs run.
        warm = spool.tile([128, 1], f32, tag="warm", bufs=1)
        nc.scalar.activation(warm[:], bias_sb[:, 0:1], AF.Exp)

        ps_tag = [f"b{i}" for i in range(8)]
        s_alt = 0       # score bank alternator (b6/b7)
        po_alt = 0      # proj-out bank alternator (b2/b3)

        def rope(acc_ps, bias_col, out_ap, tcol, uid):
            raw = spool.tile([128, SQ], bf16, tag="raw", bufs=3,
                             name=f"raw_{uid}")
            nc.scalar.activation(raw[:], acc_ps[:], AF.Identity,
                                 bias=bias_sb[:, bias_col:bias_col + 1])
            sw = spool.tile([128, SQ], bf16, tag="sw", bufs=3,
                            name=f"sw_{uid}")
            nc.vector.stream_shuffle(sw[:], raw[:], SWAP_MASK)
            nc.vector.tensor_mul(out_ap, raw[:], cos_sb[:, tcol:tcol + SQ])
            nc.vector.tensor_mul(sw[:], sw[:], sin_sb[:, tcol:tcol + SQ])
            nc.vector.tensor_add(out_ap, out_ap, sw[:])

        for b in range(B):
            rotK = kvpool.tile([128, T], bf16, tag="rotK", name=f"rotK_{b}")
            vbuf = kvpool.tile([128, T], bf16, tag="vbuf", name=f"vbuf_{b}")
            for hh in range(2):
                yts = [ypool.tile([128, HB], bf16, tag=f"yt{h}",
                                  name=f"yt{h}_{b}_{hh}") for h in range(HPC)]
                for jj in range(2):
                    j = hh * 2 + jj
                    bj = b * CPB + j
                    tcol = j * SQ

                    # ---- x halves for this chunk (prefetchable) ----
                    xh = []
                    for half in range(2):
                        t_ = xpool.tile([128, 16, SQ], bf16, tag="xh",
                                        name=f"xh_{bj}_{half}")
                        r = (bj * 2 + half) * 128
                        nc.sync.dma_start(t_.rearrange("p c n -> p (c n)"),
                                          xP_d[r:r + 128, :])
                        xh.append(t_)

                    # ---- stage A: acc-major K, V, Q0..Q3 ----
                    accs = []
                    specs = [("K", ps_tag[4], wkv_sb, 0),
                             ("V", ps_tag[5], wkv_sb, 128)]
                    specs += [(f"Q{h}", ps_tag[h], wq_sb, h * 128)
                              for h in range(HPC)]
                    for name, tag, wsb, coff in specs:
                        acc = psum.tile([128, SQ], f32, tag=tag,
                                        name=f"acc{name}_{bj}")
                        for e in range(ECH):
                            nc.tensor.matmul(
                                acc[:], wsb[:, e, coff:coff + 128],
                                xh[e // 16][:, e % 16, :],
                                start=(e == 0), stop=(e == ECH - 1))
                        accs.append(acc)
                        if name == "K":
                            rope(acc, 4, rotK[:, tcol:tcol + SQ], tcol,
                                 f"K_{bj}")
                        elif name == "V":
                            vraw = spool.tile([128, SQ], bf16, tag="raw",
                                              bufs=3, name=f"vraw_{bj}")
                            nc.scalar.activation(vraw[:], acc[:], AF.Identity,
                                                 bias=bias_sb[:, 5:6])
                            for t4 in range(4):
                                kc = 4 * j + t4
                                nc.sync.dma_start_transpose(
                                    vbuf[:, kc * 128:(kc + 1) * 128],
                                    vraw[:, t4 * 128:(t4 + 1) * 128])

                    qrots = []
                    for h in range(HPC):
                        qr = qpool.tile([128, SQ], bf16, tag=f"qrot{h}",
                                        name=f"qrot{h}_{bj}")
                        rope(accs[2 + h], h, qr[:], tcol, f"Q{h}_{bj}")
                        qrots.append(qr)

                    # ---- attention for this 512-query tile ----
                    nkc = 4 * j + 4
                    for h in range(HPC):
                        yt_ps = psum.tile([128, SQ], f32, tag=ps_tag[h % 2],
                                          name=f"ytps_{bj}_{h}")
                        sm_ps = psum.tile([1, SQ], f32, tag=ps_tag[4 + h % 2],
                                          name=f"smps_{bj}_{h}")
                        for kc in range(nkc):
                            off = 128 * max(0, kc - 4 * j)
                            s_ps = psum.tile([128, SQ], f32,
                                             tag=ps_tag[6 + s_alt],
                                             name=f"sps_{bj}_{h}_{kc}")
                            s_alt ^= 1
                            nc.tensor.matmul(
                                s_ps[:, off:], rotK[:, kc * 128:(kc + 1) * 128],
                                qrots[h][:, off:], start=True, stop=True)
                            es = spool.tile([128, SQ], bf16, tag="es", bufs=4,
                                            name=f"es_{bj}_{h}_{kc}")
                            nc.scalar.activation(es[:, off:], s_ps[:, off:],
                                                 AF.Exp, scale=INV_SQRT_D)
                            if kc >= 4 * j:
                                nc.vector.tensor_mul(es[:, off:off + 128],
                                                     es[:, off:off + 128],
                                                     tri_sb[:])
                            st, sp = (kc == 0), (kc == nkc - 1)
                            nc.tensor.matmul(
                                yt_ps[:, off:], vbuf[:, kc * 128:(kc + 1) * 128],
                                es[:, off:], start=st, stop=sp)
                            nc.tensor.matmul(sm_ps[0:1, off:], ones_sb[:],
                                             es[:, off:], start=st, stop=sp)
                        rr = spool.tile([1, SQ], f32, tag="rr", bufs=2,
                                        name=f"rr_{bj}_{h}")
                        nc.vector.reciprocal(rr[:], sm_ps[:])
                        rb = spool.tile([128, SQ], f32, tag="rb", bufs=2,
                                        name=f"rb_{bj}_{h}")
                        nc.gpsimd.partition_broadcast(rb[:], rr[:])
                        nc.vector.tensor_mul(yts[h][:, jj * SQ:(jj + 1) * SQ],
                                             yt_ps[:], rb[:])

                # ---- output projection for this half-batch (1024 toks) ----
                grow = b * T + hh * HB
                for ts8 in range(HB // 128):
                    po_sb = spool.tile([128, E], bf16, tag="po", bufs=2,
                                       name=f"po_{b}_{hh}_{ts8}")
                    for oc in range(8):
                        po_ps = psum.tile([128, SQ], f32,
                                          tag=ps_tag[2 + po_alt],
                                          name=f"pops_{b}_{hh}_{ts8}_{oc}")
                        po_alt ^= 1
                        for h in range(HPC):
                            nc.tensor.matmul(
                                po_ps[:],
                                yts[h][:, ts8 * 128:(ts8 + 1) * 128],
                                pjt_sb[:, h, oc * SQ:(oc + 1) * SQ],
                                start=(h == 0), stop=(h == HPC - 1))
                        nc.scalar.copy(po_sb[:, oc * SQ:(oc + 1) * SQ],
                                       po_ps[:])
                    nc.sync.dma_start(
                        out_d[grow + ts8 * 128:grow + (ts8 + 1) * 128, :],
                        po_sb[:])

    nc.compile()
    return nc


_PROG = None


def _prep_inputs(x, freq_cos, freq_sin, w_q_w, w_q_b, w_kv_w, w_kv_b, proj_w):
    import ml_dtypes
    bf = ml_dtypes.bfloat16

    xf = np.asarray(x, np.float32).reshape(BT, E).astype(bf)
    # [bj, t, c, p] -> [bj, c, p, t] -> [bj, half, p, e16, t] -> [(bj half p), e16*t]
    xr = xf.reshape(B * CPB, SQ, ECH, 128).transpose(0, 2, 3, 1)
    xP = np.ascontiguousarray(
        xr.reshape(B * CPB, 2, 16, 128, SQ).transpose(0, 1, 3, 2, 4)
    ).reshape(B * CPB * 2 * 128, 16 * SQ)

    cosE = np.repeat(np.asarray(freq_cos, np.float32).T, 2, axis=0)
    sinE = np.repeat(np.asarray(freq_sin, np.float32).T, 2, axis=0)
    sinS = sinE.copy()
    sinS[0::2, :] *= -1.0
    cosE = cosE.astype(bf)
    sinS = sinS.astype(bf)

    kp = np.arange(128)[:, None]
    cc = np.arange(128)[None, :]
    triM = (cc >= kp).astype(bf)
    onesC = np.ones((128, 1), bf)

    w_q_w = np.asarray(w_q_w, np.float32)
    w_kv_w = np.asarray(w_kv_w, np.float32)
    w_q_b = np.asarray(w_q_b, np.float32)
    w_kv_b = np.asarray(w_kv_b, np.float32)
    proj_w = np.asarray(proj_w, np.float32)

    in_maps = []
    for c in range(NCORES):
        wq_c = w_q_w[c * 512:(c + 1) * 512, :].astype(bf)      # [512, E]
        wqP = np.ascontiguousarray(
            wq_c.T.reshape(ECH, 128, 512).transpose(1, 0, 2)
        ).reshape(128, ECH * 512)
        kT = w_kv_w[c * D:(c + 1) * D, :].T                    # [E, 128]
        vT = w_kv_w[8 * D + c * D:8 * D + (c + 1) * D, :].T
        wkv_c = np.concatenate([kT, vT], axis=1).astype(bf)    # [E, 256]
        wkvP = np.ascontiguousarray(
            wkv_c.reshape(ECH, 128, 256).transpose(1, 0, 2)
        ).reshape(128, ECH * 256)
        pjt_c = proj_w[:, c * 512:(c + 1) * 512].T.astype(bf)  # [512, E]
        pjtP = np.ascontiguousarray(
            pjt_c.reshape(HPC, 128, E).transpose(1, 0, 2)
        ).reshape(128, HPC * E)
        biasP = np.zeros((128, 6), np.float32)
        for h in range(HPC):
            biasP[:, h] = w_q_b[c * 512 + h * 128:c * 512 + (h + 1) * 128]
        biasP[:, 4] = w_kv_b[c * D:(c + 1) * D]
        biasP[:, 5] = w_kv_b[8 * D + c * D:8 * D + (c + 1) * D]
        in_maps.append({
            "xP": xP, "wqP": wqP, "wkvP": wkvP, "pjtP": pjtP,
            "cosE": cosE, "sinS": sinS, "biasP": biasP,
            "triM": triM, "onesC": onesC,
        })
    return in_maps


def kernel(x, freq_cos, freq_sin, w_q_w, w_q_b, w_kv_w, w_kv_b, proj_w, proj_b,
           start_pos=0, **_unused):
    global _PROG
    from concourse.bass_utils import run_bass_kernel_spmd

    in_maps = _prep_inputs(x, freq_cos, freq_sin, w_q_w, w_q_b,
                           w_kv_w, w_kv_b, proj_w)

    if _PROG is None:
        _PROG = _build_program()

    res = run_bass_kernel_spmd(_PROG, in_maps, core_ids=list(range(NCORES)))
    out = np.zeros((BT, E), np.float32)
    for c in range(NCORES):
        out += res.results[c]["yp"].astype(np.float32)
    out = out + np.asarray(proj_b, np.float32)[None, :]
    return out.reshape(B, T, E)


# revision 6
# speedup vs baseline: 1.8302x; 1.2263x over previous
"""Llama GQA attention block, tensor-parallel over heads across 8 TRN2 NeuronCores.

Contract: kernel(**inputs) takes the FULL inputs of the reference
(x, freq_cos, freq_sin, w_q_w, w_q_b, w_kv_w, w_kv_b, proj_w, proj_b, start_pos)
and returns the FULL output (B, T, N_EMBD) float32.

Sharding: core c owns query heads 4c..4c+3 and KV head c, plus proj columns
c*512..(c+1)*512. Each core computes a partial projection output in bf16; the
host sums the 8 partials and adds proj_b.

v2 design notes (all-bf16 matmuls, fp32 PSUM accumulation):
- All weights (wq, wkv, proj) and rope tables resident in SBUF for the whole
  kernel; x streamed per 512-token chunk in two contiguous [128,16,512] halves.
- Stage A runs acc-major (K, V, Q0..Q3 each a dense 32-matmul accumulation)
  so the rope chains for early accumulators overlap the later matmul runs.
- RoPE pair-swap via DVE stream_shuffle (no DMA round trips).
- V transposed into [tok, d] blocks via DMA xbar transpose (off the PE).
- Causal diagonal 128-blocks are column-trimmed on score/PV/sum matmuls.
- Softmax denominator reciprocal broadcast via GpSimd partition_broadcast.
"""

import math
import numpy as np
from contextlib import ExitStack

# Problem constants (hardcoded per the harness contract).
B = 2
T = 2048
E = 4096
D = 128          # head dim
NCORES = 8
HPC = 4          # query heads per core
BT = B * T       # 4096
SQ = 512         # token chunk (matmul moving dim)
ECH = 32         # contraction chunks of 128 over E
CPB = T // SQ    # 4 tok chunks per batch
INV_SQRT_D = 1.0 / math.sqrt(D)
HB = 1024        # half-batch token span for the output projection stage

SWAP_MASK = [i ^ 1 for i in range(32)]  # pair swap within each 32-partition group


def _build_program():
    import concourse.bass as bass  # noqa: F401
    import concourse.mybir as mybir
    import concourse.tile as tile
    from concourse import bacc

    f32 = mybir.dt.float32
    bf16 = mybir.dt.bfloat16
    AF = mybir.ActivationFunctionType

    nc = bacc.Bacc("TRN2", target_bir_lowering=False, debug=False)

    xP_d = nc.dram_tensor("xP", [B * CPB * 2 * 128, 16 * SQ], bf16,
                          kind="ExternalInput")
    wq_d = nc.dram_tensor("wqP", [128, ECH * 512], bf16, kind="ExternalInput")
    wkv_d = nc.dram_tensor("wkvP", [128, ECH * 256], bf16, kind="ExternalInput")
    pjt_d = nc.dram_tensor("pjtP", [128, HPC * E], bf16, kind="ExternalInput")
    cos_d = nc.dram_tensor("cosE", [128, T], bf16, kind="ExternalInput")
    sin_d = nc.dram_tensor("sinS", [128, T], bf16, kind="ExternalInput")
    bias_d = nc.dram_tensor("biasP", [128, 6], f32, kind="ExternalInput")
    tri_d = nc.dram_tensor("triM", [128, 128], bf16, kind="ExternalInput")
    ones_d = nc.dram_tensor("onesC", [128, 1], bf16, kind="ExternalInput")
    out_d = nc.dram_tensor("yp", [BT, E], bf16, kind="ExternalOutput")

    with tile.TileContext(nc) as tc, ExitStack() as ctx:
        const = ctx.enter_context(tc.tile_pool(name="const", bufs=1))
        wpool = ctx.enter_context(tc.tile_pool(name="wpool", bufs=1))
        xpool = ctx.enter_context(tc.tile_pool(name="xpool", bufs=3))
        kvpool = ctx.enter_context(tc.tile_pool(name="kvpool", bufs=2))
        qpool = ctx.enter_context(tc.tile_pool(name="qpool", bufs=2))
        spool = ctx.enter_context(tc.tile_pool(name="spool", bufs=2))
        ypool = ctx.enter_context(tc.tile_pool(name="ypool", bufs=1))
        psum = ctx.enter_context(tc.tile_pool(name="ps", bufs=1, space="PSUM"))

        # ---- resident weights / constants ----
        # Order matters: the first stage-A matmuls need wkv + the first x
        # chunk, so those DMAs go first; pjt (4 MiB) is only needed ~130us in.
        wkv_sb = wpool.tile([128, ECH, 256], bf16, tag="wkv")
        nc.sync.dma_start(wkv_sb.rearrange("p c n -> p (c n)"), wkv_d[:, :])
        bias_sb = const.tile([128, 6], f32, tag="bias")
        nc.sync.dma_start(bias_sb[:], bias_d[:, :])

        xh_pre = []
        for half in range(2):
            t_ = xpool.tile([128, 16, SQ], bf16, tag="xh", name=f"xh_0_{half}")
            nc.sync.dma_start(t_.rearrange("p c n -> p (c n)"),
                              xP_d[half * 128:(half + 1) * 128, :])
            xh_pre.append(t_)

        wq_sb = wpool.tile([128, ECH, 512], bf16, tag="wq")
        nc.sync.dma_start(wq_sb.rearrange("p c n -> p (c n)"), wq_d[:, :])
        cos_sb = const.tile([128, T], bf16, tag="cos")
        nc.sync.dma_start(cos_sb[:], cos_d[:, :])
        sin_sb = const.tile([128, T], bf16, tag="sin")
        nc.sync.dma_start(sin_sb[:], sin_d[:, :])
        tri_sb = const.tile([128, 128], bf16, tag="tri")
        nc.sync.dma_start(tri_sb[:], tri_d[:, :])
        ones_sb = const.tile([128, 1], bf16, tag="ones")
        nc.sync.dma_start(ones_sb[:], ones_d[:, :])
        pjt_sb = wpool.tile([128, HPC, E], bf16, tag="pjt")
        nc.sync.dma_start(pjt_sb.rearrange("p h n -> p (h n)"), pjt_d[:, :])

        # Pre-warm the exp activation table while initial DMAs run.
        warm = spool.tile([128, 1], f32, tag="warm", bufs=1)
        nc.scalar.activation(warm[:], bias_sb[:, 0:1], AF.Exp)

        ps_tag = [f"b{i}" for i in range(8)]
        s_alt = 0       # score bank alternator (b6/b7)
        po_alt = 0      # proj-out bank alternator (b2/b3)

        def rope(acc_ps, bias_col, out_ap, tcol, uid):
            raw = spool.tile([128, SQ], bf16, tag="raw", bufs=3,
                             name=f"raw_{uid}")
            nc.scalar.activation(raw[:], acc_ps[:], AF.Identity,
                                 bias=bias_sb[:, bias_col:bias_col + 1])
            sw = spool.tile([128, SQ], bf16, tag="sw", bufs=3,
                            name=f"sw_{uid}")
            nc.vector.stream_shuffle(sw[:], raw[:], SWAP_MASK)
            nc.vector.tensor_mul(out_ap, raw[:], cos_sb[:, tcol:tcol + SQ])
            nc.vector.tensor_mul(sw[:], sw[:], sin_sb[:, tcol:tcol + SQ])
            nc.vector.tensor_add(out_ap, out_ap, sw[:])

        for b in range(B):
            rotK = kvpool.tile([128, T], bf16, tag="rotK", name=f"rotK_{b}")
            vbuf = kvpool.tile([128, T], bf16, tag="vbuf", name=f"vbuf_{b}")
            for hh in range(2):
                yts = [ypool.tile([128, HB], bf16, tag=f"yt{h}",
                                  name=f"yt{h}_{b}_{hh}") for h in range(HPC)]
                for jj in range(2):
                    j = hh * 2 + jj
                    bj = b * CPB + j
                    tcol = j * SQ

                    # ---- x halves for this chunk (prefetchable) ----
                    if bj == 0:
                        xh = xh_pre
                    else:
                        xh = []
                        for half in range(2):
                            t_ = xpool.tile([128, 16, SQ], bf16, tag="xh",
                                            name=f"xh_{bj}_{half}")
                            r = (bj * 2 + half) * 128
                            nc.sync.dma_start(t_.rearrange("p c n -> p (c n)"),
                                              xP_d[r:r + 128, :])
                            xh.append(t_)

                    # ---- stage A: acc-major K, V, Q0..Q3 ----
                    accs = []
                    specs = [("K", ps_tag[4], wkv_sb, 0),
                             ("V", ps_tag[5], wkv_sb, 128)]
                    specs += [(f"Q{h}", ps_tag[h], wq_sb, h * 128)
                              for h in range(HPC)]
                    for name, tag, wsb, coff in specs:
                        acc = psum.tile([128, SQ], f32, tag=tag,
                                        name=f"acc{name}_{bj}")
                        for e in range(ECH):
                            nc.tensor.matmul(
                                acc[:], wsb[:, e, coff:coff + 128],
                                xh[e // 16][:, e % 16, :],
                                start=(e == 0), stop=(e == ECH - 1))
                        accs.append(acc)
                        if name == "K":
                            rope(acc, 4, rotK[:, tcol:tcol + SQ], tcol,
                                 f"K_{bj}")
                        elif name == "V":
                            vraw = spool.tile([128, SQ], bf16, tag="raw",
                                              bufs=3, name=f"vraw_{bj}")
                            nc.scalar.activation(vraw[:], acc[:], AF.Identity,
                                                 bias=bias_sb[:, 5:6])
                            for t4 in range(4):
                                kc = 4 * j + t4
                                nc.sync.dma_start_transpose(
                                    vbuf[:, kc * 128:(kc + 1) * 128],
                                    vraw[:, t4 * 128:(t4 + 1) * 128])

                    qrots = []
                    for h in range(HPC):
                        qr = qpool.tile([128, SQ], bf16, tag=f"qrot{h}",
                                        name=f"qrot{h}_{bj}")
                        rope(accs[2 + h], h, qr[:], tcol, f"Q{h}_{bj}")
                        qrots.append(qr)

                    # ---- attention for this 512-query tile ----
                    nkc = 4 * j + 4
                    for h in range(HPC):
                        yt_ps = psum.tile([128, SQ], f32, tag=ps_tag[h % 3],
                                          name=f"ytps_{bj}_{h}")
                        sm_ps = psum.tile([1, SQ], f32, tag=ps_tag[4 + h % 2],
                                          name=f"smps_{bj}_{h}")
                        for kc in range(nkc):
                            off = 128 * max(0, kc - 4 * j)
                            s_ps = psum.tile([128, SQ], f32,
                                             tag=ps_tag[6 + s_alt],
                                             name=f"sps_{bj}_{h}_{kc}")
                            s_alt ^= 1
                            nc.tensor.matmul(
                                s_ps[:, off:], rotK[:, kc * 128:(kc + 1) * 128],
                                qrots[h][:, off:], start=True, stop=True)
                            es = spool.tile([128, SQ], bf16, tag="es", bufs=4,
                                            name=f"es_{bj}_{h}_{kc}")
                            nc.scalar.activation(es[:, off:], s_ps[:, off:],
                                                 AF.Exp, scale=INV_SQRT_D)
                            if kc >= 4 * j:
                                nc.vector.tensor_mul(es[:, off:off + 128],
                                                     es[:, off:off + 128],
                                                     tri_sb[:])
                            st, sp = (kc == 0), (kc == nkc - 1)
                            nc.tensor.matmul(
                                yt_ps[:, off:], vbuf[:, kc * 128:(kc + 1) * 128],
                                es[:, off:], start=st, stop=sp)
                            nc.tensor.matmul(sm_ps[0:1, off:], ones_sb[:],
                                             es[:, off:], start=st, stop=sp)
                        rr = spool.tile([1, SQ], f32, tag="rr", bufs=2,
                                        name=f"rr_{bj}_{h}")
                        nc.vector.reciprocal_approx_fast(rr[:], sm_ps[:])
                        rb = spool.tile([128, SQ], f32, tag="rb", bufs=2,
                                        name=f"rb_{bj}_{h}")
                        nc.gpsimd.partition_broadcast(rb[:], rr[:])
                        nc.vector.tensor_mul(yts[h][:, jj * SQ:(jj + 1) * SQ],
                                             yt_ps[:], rb[:])

                # ---- output projection for this half-batch (1024 toks) ----
                grow = b * T + hh * HB
                for ts8 in range(HB // 128):
                    po_sb = spool.tile([128, E], bf16, tag="po", bufs=2,
                                       name=f"po_{b}_{hh}_{ts8}")
                    for oc in range(8):
                        po_ps = psum.tile([128, SQ], f32,
                                          tag=ps_tag[2 + po_alt],
                                          name=f"pops_{b}_{hh}_{ts8}_{oc}")
                        po_alt ^= 1
                        for h in range(HPC):
                            nc.tensor.matmul(
                                po_ps[:],
                                yts[h][:, ts8 * 128:(ts8 + 1) * 128],
                                pjt_sb[:, h, oc * SQ:(oc + 1) * SQ],
                                start=(h == 0), stop=(h == HPC - 1))
                        nc.scalar.copy(po_sb[:, oc * SQ:(oc + 1) * SQ],
                                       po_ps[:])
                    nc.sync.dma_start(
                        out_d[grow + ts8 * 128:grow + (ts8 + 1) * 128, :],
                        po_sb[:])

    nc.compile()
    return nc


_PROG = None


def _prep_inputs(x, freq_cos, freq_sin, w_q_w, w_q_b, w_kv_w, w_kv_b, proj_w):
    import ml_dtypes
    bf = ml_dtypes.bfloat16

    xf = np.asarray(x, np.float32).reshape(BT, E).astype(bf)
    # [bj, t, c, p] -> [bj, c, p, t] -> [bj, half, p, e16, t] -> [(bj half p), e16*t]
    xr = xf.reshape(B * CPB, SQ, ECH, 128).transpose(0, 2, 3, 1)
    xP = np.ascontiguousarray(
        xr.reshape(B * CPB, 2, 16, 128, SQ).transpose(0, 1, 3, 2, 4)
    ).reshape(B * CPB * 2 * 128, 16 * SQ)

    cosE = np.repeat(np.asarray(freq_cos, np.float32).T, 2, axis=0)
    sinE = np.repeat(np.asarray(freq_sin, np.float32).T, 2, axis=0)
    sinS = sinE.copy()
    sinS[0::2, :] *= -1.0
    cosE = cosE.astype(bf)
    sinS = sinS.astype(bf)

    kp = np.arange(128)[:, None]
    cc = np.arange(128)[None, :]
    triM = (cc >= kp).astype(bf)
    onesC = np.ones((128, 1), bf)

    w_q_w = np.asarray(w_q_w, np.float32)
    w_kv_w = np.asarray(w_kv_w, np.float32)
    w_q_b = np.asarray(w_q_b, np.float32)
    w_kv_b = np.asarray(w_kv_b, np.float32)
    proj_w = np.asarray(proj_w, np.float32)

    in_maps = []
    for c in range(NCORES):
        wq_c = w_q_w[c * 512:(c + 1) * 512, :].astype(bf)      # [512, E]
        wqP = np.ascontiguousarray(
            wq_c.T.reshape(ECH, 128, 512).transpose(1, 0, 2)
        ).reshape(128, ECH * 512)
        kT = w_kv_w[c * D:(c + 1) * D, :].T                    # [E, 128]
        vT = w_kv_w[8 * D + c * D:8 * D + (c + 1) * D, :].T
        wkv_c = np.concatenate([kT, vT], axis=1).astype(bf)    # [E, 256]
        wkvP = np.ascontiguousarray(
            wkv_c.reshape(ECH, 128, 256).transpose(1, 0, 2)
        ).reshape(128, ECH * 256)
        pjt_c = proj_w[:, c * 512:(c + 1) * 512].T.astype(bf)  # [512, E]
        pjtP = np.ascontiguousarray(
            pjt_c.reshape(HPC, 128, E).transpose(1, 0, 2)
        ).reshape(128, HPC * E)
        biasP = np.zeros((128, 6), np.float32)
        for h in range(HPC):
            biasP[:, h] = w_q_b[c * 512 + h * 128:c * 512 + (h + 1) * 128]
        biasP[:, 4] = w_kv_b[c * D:(c + 1) * D]
        biasP[:, 5] = w_kv_b[8 * D + c * D:8 * D + (c + 1) * D]
        in_maps.append({
            "xP": xP, "wqP": wqP, "wkvP": wkvP, "pjtP": pjtP,
            "cosE": cosE, "sinS": sinS, "biasP": biasP,
            "triM": triM, "onesC": onesC,
        })
    return in_maps


def kernel(x, freq_cos, freq_sin, w_q_w, w_q_b, w_kv_w, w_kv_b, proj_w, proj_b,
           start_pos=0, **_unused):
    global _PROG
    from concourse.bass_utils import run_bass_kernel_spmd

    in_maps = _prep_inputs(x, freq_cos, freq_sin, w_q_w, w_q_b,
                           w_kv_w, w_kv_b, proj_w)

    if _PROG is None:
        _PROG = _build_program()

    res = run_bass_kernel_spmd(_PROG, in_maps, core_ids=list(range(NCORES)))
    out = np.zeros((BT, E), np.float32)
    for c in range(NCORES):
        out += res.results[c]["yp"].astype(np.float32)
    out = out + np.asarray(proj_b, np.float32)[None, :]
    return out.reshape(B, T, E)
